# revision 39
# baseline (speedup 1.0000x reference)
"""Trainium2 Bass kernel for nn_CoconAttention (dense transformer attention block).

Sharding: 8 cores = 4 batches x 2 head-groups (8 heads each). Each core gets
pre-transposed/sliced bf16 inputs, computes its partial output outT [1024, 896]
(transposed, pre-b_proj, bf16), and the host sums head-group pairs + transposes.

On-device layout (per core, H=8 heads as 4 pairs x 2, Dh=64, T=896, Tc=128):
  qT, kT    : feature-major bf16 [64*2, tok] pair tiles
  scores^T  : [128 keys, tok] per key-chunk c, psum; exp on ACT -> probs bf16
  probs^T   : masked via precomputed band masks (DVE mult)
  v_sb      : [keys, 8 heads, 128] bf16; hi0 heads [v(64)|1|0..], hi1 [1|0..|v(64)]
              so PV psums land at partition ranges matching aT rows and the
              denominator rows sit at partition 64 (hi0) / 0 (hi1)
  normalize : DVE recip -> PE ones-broadcast matmul -> fused DVE mul into aT
  out-proj  : per tok-chunk, accumulated over pairs; engine copy -> bf16 DMA out

Emission weaves qk-projection f-tiles between attention tok-chunks as PE filler
so the ACT exp stream (the attention-phase co-bottleneck) never starves PE.
"""
import os
import sys

import numpy as np
import ml_dtypes

try:
    import concourse.bass as bass
except ImportError:  # fresh grading dir: fall back to the repo location
    sys.path.insert(0, "/opt/trn_rl_repo")
    import concourse.bass as bass
import concourse.bacc as bacc

import concourse.tile as tile
from concourse import mybir
from concourse.bass_utils import run_bass_kernel_spmd
from contextlib import ExitStack

F32 = mybir.dt.float32
BF16 = mybir.dt.bfloat16
F32R = mybir.dt.float32r
AF = mybir.ActivationFunctionType

T, Tc, NX = 896, 128, 1024
TCH = ((0, 512), (512, 896))  # tok chunks
NPAIR = 4  # head pairs per core


def _r(ap):
    return ap.bitcast(F32R)


def _cs(c, ts):
    """Live (unmasked) column start of scores chunk c within tok range [ts,te)."""
    return max(max(0, 128 * (c - 1)), ts)


def _band_pieces(c, ts, te):
    """Mask applications for chunk c in [ts,te): (s0, e0, mask_col_offset)."""
    pieces = []
    if c >= 1:
        bs = 128 * (c - 1)  # causal half: tokens [bs, bs+128)
        s0, e0 = max(bs, ts), min(bs + 128, te)
        if s0 < e0:
            pieces.append((s0, e0, s0 - bs))
    bs = 128 * c  # anti-diagonal half: tokens [bs, bs+128)
    s0, e0 = max(bs, ts), min(bs + 128, te)
    if s0 < e0:
        pieces.append((s0, e0, 128 + (s0 - bs)))
    return pieces


def build_nc():
    nc = bacc.Bacc("TRN2", target_bir_lowering=False)

    x_h = nc.dram_tensor("xT", [NX, T], BF16, kind="ExternalInput")
    ctx_h = nc.dram_tensor("ctxP", [128, 1024], BF16, kind="ExternalInput")
    wq_h = nc.dram_tensor("w_q", [NX, 512], BF16, kind="ExternalInput")
    wk_h = nc.dram_tensor("w_k", [NX, 512], BF16, kind="ExternalInput")
    wv_h = nc.dram_tensor("w_v", [NX, 512], BF16, kind="ExternalInput")
    wkc_h = nc.dram_tensor("w_kc", [NX, 512], BF16, kind="ExternalInput")
    wvc_h = nc.dram_tensor("w_vc", [NX, 512], BF16, kind="ExternalInput")
    wpj_h = nc.dram_tensor("w_pj", [512, NX], BF16, kind="ExternalInput")
    bqkc_h = nc.dram_tensor("b_qkc", [128, 12], F32, kind="ExternalInput")
    bv_h = nc.dram_tensor("b_v", [1, 512], F32, kind="ExternalInput")
    bvc_h = nc.dram_tensor("b_vc", [1, 512], F32, kind="ExternalInput")
    mb_h = nc.dram_tensor("mband", [128, 256], BF16, kind="ExternalInput")
    out_h = nc.dram_tensor("outT", [NX, T], BF16, kind="ExternalOutput")

    with tile.TileContext(nc) as tc, ExitStack() as top:
        consts = top.enter_context(tc.tile_pool(name="consts", bufs=1))
        main = top.enter_context(tc.tile_pool(name="main", bufs=1))
        misc = top.enter_context(tc.tile_pool(name="misc", bufs=1))
        probsp = top.enter_context(tc.tile_pool(name="probsp", bufs=6))
        outp = top.enter_context(tc.tile_pool(name="outp", bufs=3))
        ps = top.enter_context(tc.tile_pool(name="ps", bufs=1, space="PSUM"))

        # ---- input DMA issues, all on SP in just-in-time transfer order ----
        # (single queue => transfers drain in exactly this order; ACT stays
        # free for the exp stream)
        wkc_sb = main.tile([128, 8, 512], BF16, name="wkc_sb")
        wkc_r = wkc_h[:, :].rearrange("(kc p) f -> p kc f", p=128)
        nc.sync.dma_start(out=wkc_sb[:, :, 0:256], in_=wkc_r[:, :, 0:256])
        ctx_sb = main.tile([128, 8, Tc], BF16, name="ctx_sb")
        nc.sync.dma_start(out=ctx_sb, in_=ctx_h[:, :].rearrange("p (kc t) -> p kc t", t=Tc))
        bias_qkc = consts.tile([128, 12], F32, name="bias_qkc")
        nc.sync.dma_start(out=bias_qkc, in_=bqkc_h[:, :])
        nc.sync.dma_start(out=wkc_sb[:, :, 256:512], in_=wkc_r[:, :, 256:512])
        wv_sbw = main.tile([128, 8, 512], BF16, name="wv_sbw")
        wv_r = wv_h[:, :].rearrange("(kc p) f -> p kc f", p=128)
        nc.sync.dma_start(out=wv_sbw[:, :, 0:256], in_=wv_r[:, :, 0:256])
        x_sb = main.tile([128, 8, T], BF16, name="x_sb")
        xr = x_h[:, :].rearrange("(kc p) t -> p kc t", p=128)
        nc.sync.dma_start(out=x_sb[:, :, 0:256], in_=xr[:, :, 0:256])
        nc.sync.dma_start(out=wv_sbw[:, :, 256:512], in_=wv_r[:, :, 256:512])
        for xs, xe in ((256, 512), (512, 768), (768, 896)):
            nc.sync.dma_start(out=x_sb[:, :, xs:xe], in_=xr[:, :, xs:xe])
        wvc_sb = main.tile([128, 8, 512], BF16, name="wvc_sb")
        nc.sync.dma_start(out=wvc_sb, in_=wvc_h[:, :].rearrange("(kc p) f -> p kc f", p=128))
        wq_sb = main.tile([128, 8, 512], BF16, name="wq_sb")
        nc.sync.dma_start(out=wq_sb, in_=wq_h[:, :].rearrange("(kc p) f -> p kc f", p=128))
        wk_sb = main.tile([128, 8, 512], BF16, name="wk_sb")
        nc.sync.dma_start(out=wk_sb, in_=wk_h[:, :].rearrange("(kc p) f -> p kc f", p=128))
        maskband = consts.tile([128, 256], BF16, name="maskband")
        nc.sync.dma_start(out=maskband, in_=mb_h[:, :])
        wpj_sb = main.tile([128, 4, 1024], BF16, name="wpj_sb")
        nc.sync.dma_start(out=wpj_sb, in_=wpj_h[:, :].rearrange("(kc p) o -> p kc o", p=128))

        # ---- small constants ----
        ebias = consts.tile([128, 2], F32, name="ebias")  # exp bias: [0]=0, [1]=ctx -2
        nc.vector.memset(ebias[:, 0:1], 0.0)
        nc.vector.memset(ebias[:, 1:2], -2.0)
        ones = consts.tile([128, 64], BF16, name="ones")
        nc.vector.memset(ones, 1.0)

        # ---- persistent activation tiles ----
        qT = [main.tile([128, T], BF16, name=f"qT{p}") for p in range(NPAIR)]
        kT = [main.tile([128, Tc + T], BF16, name=f"kT{p}") for p in range(NPAIR)]
        aT = [main.tile([128, T], BF16, name=f"aT{p}") for p in range(NPAIR)]
        # v_sb[c]: per head h: hi0 (even h): [v(64) | ones(1) | 0 x63]
        #          hi1 (odd h):  [ones(1) | 0 x63 | v(64)]
        v_sb = [main.tile([128, 8, 128], BF16, name=f"v{c}") for c in range(8)]
        for c in range(8):
            v4 = v_sb[c].rearrange("p (q hi) d -> p q hi d", hi=2)
            nc.gpsimd.memset(v4[:, :, 1, 1:64], 0.0)   # hi1 junk cols -> 0
            nc.gpsimd.memset(v4[:, :, 0, 64:65], 1.0)  # hi0 denominator ones
            nc.gpsimd.memset(v4[:, :, 1, 0:1], 1.0)    # hi1 denominator ones

        # v/vc bias rows broadcast to all partitions (stride-0 DRAM read, SWDGE);
        # issued after the memsets so their transfers don't cut the input queue
        bvb = consts.tile([128, 512], F32, name="bvb")
        nc.gpsimd.dma_start(out=bvb, in_=bv_h[:, :].partition_broadcast(128))
        bvcb = consts.tile([128, 512], F32, name="bvcb")
        nc.gpsimd.dma_start(out=bvcb, in_=bvc_h[:, :].partition_broadcast(128))

        def vsplit(ap512):
            """[128, 512] feature AP -> [128, 4, 2, 64] (pair, hi, dh)."""
            return ap512.rearrange("p (q hi d) -> p q hi d", hi=2, d=64)

        with ExitStack() as ph1:
            # ---- ctx projections (kcT + vc) ----
            for f in range(4):
                pt = ps.tile([128, 512], F32, tag="gen", bufs=2, name=f"pkc{f}")
                for kc in range(8):
                    nc.tensor.matmul(
                        pt[:, 0:Tc], wkc_sb[:, kc, 128 * f:128 * f + 128],
                        ctx_sb[:, kc, :], start=(kc == 0), stop=(kc == 7))
                nc.scalar.activation(
                    out=kT[f][:, 0:Tc], in_=pt[:, 0:Tc], func=AF.Identity,
                    bias=bias_qkc[:, 8 + f:9 + f], scale=1.0)
            # ---- v projection (before vc: wvc arrives later in the DMA queue;
            # feature halves so tt0 starts on the first wv half-DMA) ----
            for tt in range(7):
                pt = ps.tile([128, 512], F32, tag="gen", bufs=2, name=f"pv{tt}")
                for fh in (0, 256):
                    for kc in range(8):
                        nc.tensor.matmul(
                            pt[:, fh:fh + 256], x_sb[:, kc, 128 * tt:128 * tt + 128],
                            wv_sbw[:, kc, fh:fh + 256], start=(kc == 0),
                            stop=(kc == 7), skip_group_check=True)
                v4 = v_sb[1 + tt].rearrange("p (q hi) d -> p q hi d", hi=2)
                for hi in range(2):
                    nc.vector.tensor_add(
                        out=v4[:, :, hi, 64 * hi:64 * hi + 64],
                        in0=vsplit(pt[:, 0:512])[:, :, hi, :],
                        in1=vsplit(bvb)[:, :, hi, :])

            # ---- vc projection ----
            pt = ps.tile([128, 512], F32, tag="gen", bufs=2, name="pvc")
            for kc in range(8):
                nc.tensor.matmul(
                    pt[:, 0:512], ctx_sb[:, kc, :], wvc_sb[:, kc, :],
                    start=(kc == 0), stop=(kc == 7))
            v4 = v_sb[0].rearrange("p (q hi) d -> p q hi d", hi=2)
            for hi in range(2):
                nc.vector.tensor_add(
                    out=v4[:, :, hi, 64 * hi:64 * hi + 64],
                    in0=vsplit(pt[:, 0:512])[:, :, hi, :],
                    in1=vsplit(bvcb)[:, :, hi, :])

            # ---- qT / kT projections (transposed layout), per pair ----
            def qk_unit(w_sb, f, dest, dcol, bias_col, tch):
                ts, te = tch
                pt = ps.tile([128, 512], F32, tag="gen", bufs=2,
                             name=f"pqk{bias_col}{ts}")
                for kc in range(8):
                    nc.tensor.matmul(
                        pt[:, 0:te - ts], w_sb[:, kc, 128 * f:128 * f + 128],
                        x_sb[:, kc, ts:te], start=(kc == 0), stop=(kc == 7))
                nc.vector.tensor_scalar_add(
                    out=dest[:, dcol + ts:dcol + te], in0=pt[:, 0:te - ts],
                    scalar1=bias_qkc[:, bias_col:bias_col + 1])

            def qk_ftile(w_sb, f, dest, dcol, bias_col):
                for tch in TCH:
                    qk_unit(w_sb, f, dest, dcol, bias_col, tch)

            def sc_chunk(p, t_i, c):
                """Emit scores matmuls + exp + masks for chunk c; returns (c, cs, pb)."""
                ts, te = TCH[t_i]
                cs = _cs(c, ts)
                sc = ps.tile([128, 2, 512], F32, tag="sc", bufs=2,
                             name=f"sc{p}{t_i}{c}")
                for hi in range(2):
                    nc.tensor.matmul(
                        sc[:, hi, cs - ts:te - ts],
                        kT[p][64 * hi:64 * hi + 64, 128 * c:128 * c + 128],
                        qT[p][64 * hi:64 * hi + 64, cs:te],
                        start=True, stop=True, tile_position=(64 * hi, 0))
                pb = probsp.tile([128, 2, 512], BF16, tag="pb", name=f"pb{p}{t_i}{c}")
                nc.scalar.activation(
                    out=pb[:, :, cs - ts:te - ts], in_=sc[:, :, cs - ts:te - ts],
                    func=AF.Exp,
                    bias=(ebias[:, 1:2] if c == 0 else ebias[:, 0:1]),
                    scale=0.125)
                for hi in range(2):
                    for s0, e0, mc in _band_pieces(c, ts, te):
                        nc.vector.tensor_mul(
                            out=pb[:, hi, s0 - ts:e0 - ts],
                            in0=pb[:, hi, s0 - ts:e0 - ts],
                            in1=maskband[:, mc:mc + (e0 - s0)])
                return (c, cs, pb)

            def pv(p, t_i, at1, at2, c, cs, pb, last_c):
                ts, te = TCH[t_i]
                h0, h1 = 2 * p, 2 * p + 1
                nc.tensor.matmul(
                    at1[0:65, cs - ts:te - ts], v_sb[c][:, h0, 0:65],
                    pb[:, 0, cs - ts:te - ts],
                    start=(c == 0), stop=(c == last_c), skip_group_check=True)
                nc.tensor.matmul(
                    at2[0:128, cs - ts:te - ts], v_sb[c][:, h1, 0:128],
                    pb[:, 1, cs - ts:te - ts],
                    start=(c == 0), stop=(c == last_c), skip_group_check=True)

            def attn_t0(p, mid=None):
                """Token chunk 0: 2-deep score/PV pipeline; returns at tiles.
                `mid` (prev pair's delayed t1-norm) is emitted after the first
                two score chunks so its DVE work queues behind this pair's
                first masks instead of blocking them."""
                queue = [sc_chunk(p, 0, 0), sc_chunk(p, 0, 1)]
                if mid is not None:
                    mid()
                at1 = ps.tile([65, 512], F32, tag="at1", bufs=1, name=f"at1_{p}0")
                at2 = ps.tile([128, 512], F32, tag="at2", bufs=1, name=f"at2_{p}0")
                for c in range(2, 5):
                    queue.append(sc_chunk(p, 0, c))
                    pv(p, 0, at1, at2, *queue.pop(0), 4)
                while queue:
                    pv(p, 0, at1, at2, *queue.pop(0), 4)
                return at1, at2

            def attn_t1_rest(p, queue, fillers=()):
                """Emit remaining t1 chunks, consuming the prefix queue.
                `fillers` are PE work units interleaved after each PV step to
                keep PE fed while ACT/DVE drain the exp/mask backlog."""
                fillers = list(fillers)
                at1 = ps.tile([65, 512], F32, tag="at1", bufs=1, name=f"at1_{p}1")
                at2 = ps.tile([128, 512], F32, tag="at2", bufs=1, name=f"at2_{p}1")
                for c in range(len(queue), 8):
                    queue.append(sc_chunk(p, 1, c))
                    pv(p, 1, at1, at2, *queue.pop(0), 7)
                    if fillers:
                        fillers.pop(0)()
                while queue:
                    pv(p, 1, at1, at2, *queue.pop(0), 7)
                    if fillers:
                        fillers.pop(0)()
                for f in fillers:
                    f()
                return at1, at2

            def norm(p, t_i, at1, at2):
                """Pool copy of denom rows -> PE ones-broadcast (bf16) ->
                full-width DVE reciprocal -> muls into aT (per tchunk)."""
                ts, te = TCH[t_i]
                w2 = te - ts
                rd = misc.tile([128, 512], BF16, tag="rd", bufs=2, name=f"rd{p}{t_i}")
                nc.vector.tensor_copy(out=rd[64:65, 0:w2], in_=at1[64:65, 0:w2])
                nc.vector.tensor_copy(out=rd[0:1, 0:w2], in_=at2[0:1, 0:w2])
                dnb = ps.tile([128, 512], F32, tag="gen", bufs=2, name=f"dnb{p}{t_i}")
                nc.tensor.matmul(
                    dnb[0:64, 0:w2], ones[64:65, 0:64], rd[64:65, 0:w2],
                    start=True, stop=True, skip_group_check=True)
                nc.tensor.matmul(
                    dnb[64:128, 0:w2], ones[0:1, 0:64], rd[0:1, 0:w2],
                    start=True, stop=True, skip_group_check=True)
                rbs = misc.tile([128, 512], F32, tag="rbs", bufs=2, name=f"rb{p}{t_i}")
                nc.vector.reciprocal(out=rbs[:, 0:w2], in_=dnb[:, 0:w2])
                nc.vector.tensor_mul(
                    out=aT[p][0:64, ts:te], in0=at1[0:64, 0:w2], in1=rbs[0:64, 0:w2])
                nc.vector.tensor_mul(
                    out=aT[p][64:128, ts:te], in0=at2[64:128, 0:w2],
                    in1=rbs[64:128, 0:w2])

            ob_t = [outp.tile([128, 8, 512], BF16, tag=f"ob{t}", name=f"ob{t}")
                    for t in range(2)]
            out_r = out_h[:, :].rearrange("(of p) t -> p of t", p=128)

            def po_unit(t_i, of):
                ts, te = TCH[t_i]
                w2 = te - ts
                # tail tiles alternate into the (by then idle) "sc" slots for a
                # deeper psum pipeline
                tag = "gen" if (t_i == 0 or of % 2 == 0) else "sc"
                po = ps.tile([128, 512], F32, tag=tag, bufs=2, name=f"po{of}{t_i}")
                for kp in range(4):
                    nc.tensor.matmul(
                        po[:, 0:w2], wpj_sb[:, kp, 128 * of:128 * of + 128],
                        aT[kp][:, ts:te], start=(kp == 0), stop=(kp == 3),
                        skip_group_check=True)
                ob = ob_t[t_i][:, of, :]
                # copies alternate ACT/DVE (GPSIMD can't read PSUM)
                if of % 2 == 0:
                    nc.scalar.copy(out=ob[:, 0:w2], in_=po[:, 0:w2])
                else:
                    nc.vector.tensor_copy(out=ob[:, 0:w2], in_=po[:, 0:w2])
                # batched DMAs (HWDGE gen is 625ns each); tail staggered so the
                # final transfer is short
                if t_i == 0:
                    if of == 7:
                        nc.sync.dma_start(out=out_r[:, :, ts:te],
                                          in_=ob_t[0][:, :, 0:w2])
                elif of in (3, 5, 6, 7):
                    o0 = {3: 0, 5: 4, 6: 6, 7: 7}[of]
                    nc.sync.dma_start(out=out_r[:, o0:of + 1, ts:te],
                                      in_=ob_t[1][:, o0:of + 1, 0:w2])

            # ---- emission: weave qk f-tiles / out-proj between attention ----
            qk_ftile(wq_sb, 0, qT[0], 0, 0)
            qk_ftile(wk_sb, 0, kT[0], Tc, 4)
            pend_norm = None
            for p in range(NPAIR):
                a0 = attn_t0(p, mid=pend_norm)
                if p < NPAIR - 1:
                    qk_ftile(wq_sb, p + 1, qT[p + 1], 0, p + 1)
                t1q = [sc_chunk(p, 1, c) for c in range(5)]
                norm(p, 0, *a0)
                if p < NPAIR - 1:
                    fillers = [
                        (lambda tch=tch: qk_unit(wk_sb, p + 1, kT[p + 1], Tc,
                                                 4 + p + 1, tch))
                        for tch in TCH]
                else:
                    fillers = [(lambda of=of: po_unit(0, of)) for of in range(8)]
                a1 = attn_t1_rest(p, t1q, fillers)
                if p < NPAIR - 1:
                    pend_norm = (lambda p=p, a1=a1: norm(p, 1, *a1))
                else:
                    norm(p, 1, *a1)
            for of in range(8):
                po_unit(1, of)

    if not nc.is_finalized():
        nc.finalize()
    return nc


_NC_CACHE = {}


def _get_nc():
    if "nc" not in _NC_CACHE:
        _NC_CACHE["nc"] = build_nc()
    return _NC_CACHE["nc"]


def _pack128(v):
    """[128*n] -> [128, n] with [p, f] = v[128*f + p]."""
    n = v.shape[0] // 128
    return np.ascontiguousarray(v.reshape(n, 128).T)


def make_in_maps(inputs):
    bf16 = ml_dtypes.bfloat16
    x = np.asarray(inputs["x"], np.float32)
    ctx_seq = np.asarray(inputs["context_seq"], np.float32)
    w_ref = np.asarray(inputs["w_ref"], np.float32)
    b_ref = np.asarray(inputs["b_ref"], np.float32)
    w_attn = np.asarray(inputs["w_attn"], np.float32)
    b_attn = np.asarray(inputs["b_attn"], np.float32)
    w_proj = np.asarray(inputs["w_proj"], np.float32)

    # mask band constant: cols 0-127 causal (1 where q>=p), cols 128-255
    # anti-diagonal (0 where q==p else 1)
    qq = np.arange(128)[None, :]
    pp = np.arange(128)[:, None]
    mband = np.concatenate([(qq >= pp), (qq != pp)], axis=1).astype(bf16)
    mband = np.ascontiguousarray(mband)

    in_maps = []
    for b in range(4):
        xT = np.ascontiguousarray(x[b].T.astype(bf16))
        # ctx pre-shuffled so the on-device [128, 8, Tc] load is a straight copy:
        # ctxP[p, kc*Tc + t] = ctx[t, 128*kc + p]
        ctxT = ctx_seq[b].T.astype(bf16)  # [NX, Tc]
        ctxP = np.ascontiguousarray(
            ctxT.reshape(8, 128, Tc).transpose(1, 0, 2).reshape(128, 8 * Tc))
        for g in range(2):
            sl = slice(512 * g, 512 * g + 512)
            b_qkc = _pack128(np.concatenate(
                [b_attn[0 * NX:1 * NX][sl], b_attn[1 * NX:2 * NX][sl],
                 b_ref[0 * NX:1 * NX][sl]]))
            in_maps.append(dict(
                xT=xT,
                ctxP=ctxP,
                w_q=np.ascontiguousarray(w_attn[:, 0 * NX:1 * NX][:, sl].astype(bf16)),
                w_k=np.ascontiguousarray(w_attn[:, 1 * NX:2 * NX][:, sl].astype(bf16)),
                w_v=np.ascontiguousarray(w_attn[:, 2 * NX:3 * NX][:, sl].astype(bf16)),
                w_kc=np.ascontiguousarray(w_ref[:, 0 * NX:1 * NX][:, sl].astype(bf16)),
                w_vc=np.ascontiguousarray(w_ref[:, 1 * NX:2 * NX][:, sl].astype(bf16)),
                w_pj=np.ascontiguousarray(w_proj[sl, :].astype(bf16)),
                b_qkc=b_qkc,
                b_v=np.ascontiguousarray(b_attn[2 * NX:3 * NX][sl].reshape(1, 512)),
                b_vc=np.ascontiguousarray(b_ref[1 * NX:2 * NX][sl].reshape(1, 512)),
                mband=mband,
            ))
    return in_maps


def kernel(**inputs):
    b_proj = np.asarray(inputs["b_proj"], np.float32)
    in_maps = make_in_maps(inputs)
    nc = _get_nc()
    res = run_bass_kernel_spmd(nc, in_maps, core_ids=list(range(8)),
                               trace=os.environ.get("COCON_TRACE", "") == "1")
    outs = res.results
    out = np.empty((4, T, NX), np.float32)
    for b in range(4):
        acc = (outs[2 * b]["outT"].astype(np.float32)
               + outs[2 * b + 1]["outT"].astype(np.float32))  # [1024, 896]
        out[b] = acc.T + b_proj[None, :]
    if res.exec_time_ns is not None:
        kernel.last_exec_time_ns = res.exec_time_ns
    return out


kernel.last_exec_time_ns = None
